# revision 5
# baseline (speedup 1.0000x reference)
"""Trainium2 Bass kernel for nn_RNN_Tensorized (SWDGE trigger store).

Math: in the reference model, layers 2 and 3 receive sigma == zeros, so their
bilinear terms vanish exactly and the output collapses (exact algebra, any
input values) to a batch-constant row:

    out[b, l] = sigmoid( sum_h elu(b3[l,h]) * (Ws[l,h,1]-Ws[l,h,0])
                         + bs[l,1]-bs[l,0] )

Sharding is pure data parallelism over batch: each of the 8 cores computes
the same 64-vector f and writes its own [1024, 64] shard. Host-side packing
(elu(b3)*wd products, bias folded as a double-bf16 hi/lo pair carrying the
products' bf16 rounding residue) is weight folding; the device does the
cross-element reduce (66-row PE matmul against the preamble's ones column),
the Sigmoid ACT, and all IO.

IO strategy, built around the fixed DMA overheads of the TRN2 cost model:

  - The [1024, 64] bf16 store no longer uses an HWDGE dma_start (whose 625ns
    descriptor-gen + 650ns DGE->DMA delay sit on the critical path after the
    activation). Instead the store is a KV-writeback SWDGE DMA in
    PREPARE_ONLY mode: the Pool engine generates all descriptors (~1us) in
    the shadow of the input DMA, and after the activation completes a tiny
    trigger_dma fires the pre-built ring (no HWDGE, no DGE delay).

  - Store geometry: out[1024, 64] viewed as KV cache [batch=1, dhi=128,
    dho=8, n_ctx=64] with ctx_idx=0 and ncn=64, so chunk (i,o) of 64
    elements lands at rows 8i+o. The SBUF source AP uses a 0-stride dho
    axis, so the engine reads the same one-row f tile [128, 64] for all 8
    row-copies per partition — the ACT writes only 64 elements/partition.

  - The activation output tile is read by descriptor address only at prep
    time; the trigger waits on the ACT completion semaphore before firing,
    and a trailing wait_ge on the store's SDMA semaphore keeps the program
    alive until the data has landed in DRAM.
"""

import numpy as np

import concourse.bass as bass
from concourse import bacc, mybir
from concourse.bass_utils import run_bass_kernel_spmd

N_CORES = 8
B, L, H = 8192, 64, 64
B_SHARD = B // N_CORES  # 1024
K = H + 2  # 66 reduce rows: 64 h-rows + bias hi/lo rows (double-bf16)

# KV-writeback store geometry: out [1024, 64] = [batch=1, dhi=128, dho=8,
# n_ctx=64], d_head = dhi*dho = 1024, ncn = n_ctx = 64 (full row per chunk).
DHI, DHO, NCTX, NCN = 128, 8, 64, 64

F32 = mybir.dt.float32
BF16 = mybir.dt.bfloat16
I32 = mybir.dt.int32
ACTF = mybir.ActivationFunctionType


def build_kernel():
    nc = bacc.Bacc(enable_partition_id=False, monotonic_sem_count=0)
    pk = nc.declare_dram_parameter("pk", [K, L], BF16, isOutput=False)
    out = nc.declare_dram_parameter("out", [B_SHARD, L], BF16, isOutput=True)

    from contextlib import ExitStack

    with ExitStack() as ctx:
        tP = ctx.enter_context(nc.sbuf_tensor([K, L], BF16))
        fvec = ctx.enter_context(nc.sbuf_tensor([128, L], BF16))
        warm = ctx.enter_context(nc.sbuf_tensor([1, 1], F32))
        psum = ctx.enter_context(nc.psum_tensor([128, L], F32))
        dma_sem = ctx.enter_context(nc.semaphore("dma_sem"))
        mm_sem = ctx.enter_context(nc.semaphore("mm_sem"))
        act_sem = ctx.enter_context(nc.semaphore("act_sem"))
        prep_sem = ctx.enter_context(nc.semaphore("prep_sem"))
        st_sem = ctx.enter_context(nc.semaphore("st_sem"))
        w_sem = ctx.enter_context(nc.semaphore("w_sem"))

        # input DMA on SP HWDGE, issued at t=0
        nc.sync.dma_start(out=tP[:], in_=pk[:]).then_inc(dma_sem, 16)

        # Pool engine: the store descriptor prep (Bacc's
        # insert_library_loads pass auto-inserts the attn ucode library
        # reload the KV writeback needs). The matmul's ones column and the
        # zero ctx indices reuse the framework preamble's const tiles
        # (bf16 1.0 / f32 0.0, barrier-synchronized before any of this).
        ctxi = nc.const_aps.tensor(0.0, (128, 1), F32).bitcast(I32)

        # kv_writeback out view: [batch=1, dhi=128, dho=8, n_ctx=64]
        ov = out[:, :]
        out_kv = bass.AP(
            tensor=ov.tensor,
            offset=ov.offset,
            ap=[
                [B_SHARD * L, 1],  # batch
                [DHO * NCTX, DHI],  # dhi: 512-elem stride
                [NCTX, DHO],  # dho: 64-elem stride
                [1, NCTX],  # n_ctx contiguous
            ],
        )
        # SBUF source view: [dhi=128, dho=8 (0-stride), batch=1, ncn=64] —
        # every dho copy reads the same 64-element f row per partition.
        fv = fvec[:, :]
        in_kv = bass.AP(
            tensor=fv.tensor,
            offset=fv.offset,
            ap=[fv.ap[0], [0, DHO], [0, 1], [1, NCN]],
        )
        nc.gpsimd.kv_writeback(
            out_kv, in_kv, ctxi, prepare_only=True, sem=st_sem
        ).then_inc(prep_sem, 1)

        # psum[m, l] = sum_k ones[k] * tP[k, l] = d[l] on all 128 partitions
        onesrep = nc.const_aps.tensor(1.0, (K, 128), BF16)
        nc.tensor.matmul(psum[:], onesrep, tP[:])._wait_ge(dma_sem, 16).then_inc(
            mm_sem, 1
        )

        # prewarm the sigmoid table (no-op in sim, real on HW), then the
        # actual sigmoid: psum [128, 64] -> fvec [128, 64] bf16
        nc.vector.memset(warm[:], 0.0).then_inc(w_sem, 1)
        nc.scalar.activation(warm[:], warm[:], ACTF.Sigmoid)._wait_ge(w_sem, 1)
        nc.scalar.activation(fvec[:, :], psum[:], ACTF.Sigmoid)._wait_ge(
            mm_sem, 1
        ).then_inc(act_sem, 1)

        # fire the pre-built store descriptors once f is in SBUF
        nc.gpsimd.wait_ge(prep_sem, 1)
        nc.gpsimd.trigger_dma(1)._wait_ge(act_sem, 1)
        # keep the program alive until the store's SDMA completion lands
        nc.sync.wait_ge(st_sem, 16)

    nc.finalize()
    return nc


_NC_CACHE = None


def _pack(inputs) -> np.ndarray:
    import ml_dtypes

    bf16 = ml_dtypes.bfloat16
    b3 = np.asarray(inputs["b3"], dtype=np.float32)
    Ws = np.asarray(inputs["Ws"], dtype=np.float32)
    bs = np.asarray(inputs["bs"], dtype=np.float32)
    wd = Ws[:, :, 1] - Ws[:, :, 0]  # [L, H]
    elu = np.where(b3 > 0, b3, np.expm1(np.minimum(b3, 0.0)))  # [L, H]
    P0 = (elu * wd).T.astype(np.float32)  # [H, L]
    Pb = P0.astype(bf16)  # rounded products (exact in f32 psum accumulation)
    # Fold the total bf16 rounding error into the bias, carried as a
    # double-bf16 hi/lo pair: the device-side sum then matches the f32
    # result to ~1e-5, so only the bf16 output rounding (~2^-9) remains.
    bias = (bs[:, 1] - bs[:, 0]) - (Pb.astype(np.float32) - P0).sum(axis=0)
    hi = bias.astype(bf16)
    lo = (bias - hi.astype(np.float32)).astype(bf16)
    P = np.zeros((K, L), dtype=bf16)
    P[0:H, :] = Pb
    P[H, :] = hi
    P[H + 1, :] = lo
    return P


def kernel(**inputs) -> np.ndarray:
    global _NC_CACHE
    packed = _pack(inputs)
    # Host-side check value for transient-corruption detection: the device
    # computes sigmoid of the f32 column sums of `packed`; bf16 IO bounds the
    # device-vs-host deviation to ~4e-3, far under the validation threshold.
    d = packed.astype(np.float32).sum(axis=0)
    f_ref = 1.0 / (1.0 + np.exp(-d))
    in_maps = [{"pk": packed} for _ in range(N_CORES)]
    # The PJRT execute path rarely fails transiently (LoadExecutable /
    # device-state hiccup), and has once been seen returning corrupted data
    # without raising. Retry with a freshly built module on either symptom;
    # always return device-produced data.
    last_err, last_out = None, None
    for attempt in range(3):
        if _NC_CACHE is None:
            _NC_CACHE = build_kernel()
        try:
            res = run_bass_kernel_spmd(
                _NC_CACHE, in_maps, core_ids=list(range(N_CORES))
            )
            shards = [
                np.asarray(res.results[i]["out"]).astype(np.float32)
                for i in range(N_CORES)
            ]
            out_full = np.concatenate(shards, axis=0)
        except Exception as e:  # noqa: BLE001 - retry, then surface
            last_err = e
            _NC_CACHE = None
            continue
        last_out = out_full
        if np.abs(out_full - f_ref[None, :]).max() <= 1e-2:
            return out_full
        _NC_CACHE = None  # corrupted run: rebuild and retry
    if last_out is not None:
        return last_out
    raise last_err
